# revision 9
# baseline (speedup 1.0000x reference)
"""Trainium2 Bass kernel for nn_LDRFat (3-layer MLP forward).

reference: logits = relu((x @ W) @ fc_w.T + fc_b) @ logits_w.T + logits_b

Algebraic optimization: (x @ W) @ fc_w.T == x @ (W @ fc_w.T).
Precomputing Wfc = W @ fc_w.T ([3072,512]) collapses the dominant
309-GFLOP x@W matmul into a 51.5-GFLOP x@Wfc (phase B).

Device strategy (all matmul operands bf16, f32 accumulate):
 - Host feeds pre-transposed, pre-permuted operands laid out exactly as
   the SBUF tiles ([partition, free] contiguous): full-bandwidth DMAs,
   zero device transposes.
 - PE warm-up: ~30 throwaway matmuls at t=0 so the HAM clock-gate
   reaches 2.4 GHz before phase A.
 - Phase A (sharded over W's k-rows, 3 k-tiles/core): wfc_shard =
   W[kshard,:] @ fc_w.T, nt-outer with 3 open PSUM groups; its inputs
   (5.5 MB) are DMA'd in 4 chunks on the sync queue so compute starts
   as soon as the first lands.
 - One bf16 AllGather combines the wfc shards. Measured behavior: the
   collective's wire time cannot overlap bulk SDMA traffic (same 16
   engines), so the big xT load is deliberately queued on the gpsimd
   (SWDGE) queue BEHIND collective_compute — the gather runs on a
   quiet fabric right after phase A (~30 us in), and the four
   mc-chunks of xT stream in behind it.
 - Phase B mc-major: for each m-chunk, 4 f-groups of 24 accumulating
   matmuls (stationary wfc tile, moving xT, free=512). The first group
   only needs xT chunk 0, so phase B starts while chunks 1-3 are still
   in flight; each 25-us mc-block outlasts the 9-us chunk arrival.
   ACT applies fused bias+relu; per-chunk logits (outT[cls,m],
   logits_wT stationary, bias via K=1 ones x logits_b matmul) overlap
   the next chunk's matmuls. Host transposes the [10, 2048] result.
"""

import os
import numpy as np
import ml_dtypes

import concourse.bass as bass
import concourse.mybir as mybir
import concourse.tile as tile
from concourse import bacc
from concourse.bass import MemorySpace, ts, ds
from concourse.bass_utils import run_bass_kernel_spmd

B = 16384
N = 3072
FC = 512
CLS = 10
NCORES = 8
BS = B // NCORES     # 2048 batch rows per core
P = 128

KT = N // P          # 24 k/n tiles
FT = FC // P         # 4 f-tiles
MC = 4               # m-chunks per core
MCH = BS // MC       # 512
KSH = KT // NCORES   # 3 k-tiles per core in sharded precompute
WK = KSH * P         # 384 W-rows per core
NCHUNK = 4           # phase-A input DMA chunks
NTC = KT // NCHUNK   # nt-tiles per chunk
NWARM = 30           # PE warm-up matmuls

F32 = mybir.dt.float32
BF16 = mybir.dt.bfloat16
BF = ml_dtypes.bfloat16

_CACHE = {}
LAST_RESULT = None


def build_kernel():
    nc = bacc.Bacc(
        "TRN2",
        target_bir_lowering=False,
        debug=False,
        enable_asserts=False,
        num_devices=NCORES,
    )
    # pre-permuted [partition, free] layouts (see prep_inputs); xTr is
    # mc-major: MC blocks of [P, KT*MCH]
    xT_d = nc.dram_tensor("xTr", [P, MC * KT * MCH], BF16, kind="ExternalInput").ap()
    wts_d = nc.dram_tensor("WTsr", [P, KT * WK], BF16, kind="ExternalInput").ap()
    fcwT_d = nc.dram_tensor("fcwTr", [P, KT * FC], BF16, kind="ExternalInput").ap()
    fcb_d = nc.dram_tensor("fc_b", [FC], F32, kind="ExternalInput").ap()
    lgwT_d = nc.dram_tensor("lgwT", [FC, CLS], BF16, kind="ExternalInput").ap()
    lgb_d = nc.dram_tensor("lgb", [CLS], BF16, kind="ExternalInput").ap()
    out_d = nc.dram_tensor("out", [CLS, BS], F32, kind="ExternalOutput").ap()

    with tile.TileContext(nc) as tc:
        with (
            tc.tile_pool(name="consts", bufs=1) as consts,
            tc.tile_pool(name="wfc", bufs=1) as wfc_pool,
            tc.tile_pool(name="xt", bufs=1) as xt_pool,
            tc.tile_pool(name="yt", bufs=2) as yt_pool,
            tc.tile_pool(name="osb", bufs=1) as out_pool,
        ):
            # ---- constants / small inputs (issued first on sync queue) ----
            fcb_sb = consts.tile([P, FT], F32)
            nc.sync.dma_start(fcb_sb, fcb_d.rearrange("(t p) -> p t", p=P))
            lgwT_sb = consts.tile([P, FT, CLS], BF16)
            nc.sync.dma_start(lgwT_sb, lgwT_d.rearrange("(t p) c -> p t c", p=P))
            lgb_sb = consts.tile([1, CLS], BF16)
            nc.sync.dma_start(lgb_sb, lgb_d.rearrange("(a c) -> a c", a=1))
            ones_stage = consts.tile([1, MCH], F32)
            nc.gpsimd.memset(ones_stage, 1.0)
            ones_sb = consts.tile([1, MCH], BF16)
            nc.vector.tensor_copy(ones_sb, ones_stage)

            # ---- phase-A operands, chunked, on the sync queue ----
            wts_sb = consts.tile([P, KT, WK], BF16)
            fcwT_sb = consts.tile([P, KT, FC], BF16)
            for c in range(NCHUNK):
                nc.sync.dma_start(
                    wts_sb[:, ts(c, NTC)].rearrange("p t k -> p (t k)"),
                    wts_d[:, ds(c * NTC * WK, NTC * WK)],
                )
                nc.sync.dma_start(
                    fcwT_sb[:, ts(c, NTC)].rearrange("p t f -> p (t f)"),
                    fcwT_d[:, ds(c * NTC * FC, NTC * FC)],
                )

            wfc_sb = wfc_pool.tile([P, KT, FC], BF16)
            xts = [
                xt_pool.tile([P, KT, MCH], BF16, tag=f"xt{mc}", name=f"xt{mc}")
                for mc in range(MC)
            ]

            # ---------------- Phase A: wfc = W @ fc_w.T ----------------
            with (
                tc.tile_pool(name="wstg", bufs=1) as stage_pool,
                tc.tile_pool(name="ps_a", bufs=3, space=MemorySpace.PSUM) as ps_a,
                tc.tile_pool(name="ps_wu", bufs=1, space=MemorySpace.PSUM) as ps_wu,
                tc.tile_pool(name="ccd", bufs=1, space=MemorySpace.DRAM) as ccd,
            ):
                # PE warm-up while input DMAs are in flight
                wu_w = stage_pool.tile([P, P], BF16)
                nc.gpsimd.memset(wu_w, 0.0)
                wu_x = stage_pool.tile([P, MCH], BF16)
                nc.gpsimd.memset(wu_x, 0.0)
                wu_ps = ps_wu.tile([P, MCH], F32)
                for _ in range(NWARM):
                    nc.tensor.matmul(wu_ps, wu_w, wu_x, start=True, stop=True)

                wfc_stage = stage_pool.tile([P, KSH, FC], BF16)
                accs = [
                    ps_a.tile([P, FC], F32, tag="acc", name=f"acc{lkt}")
                    for lkt in range(KSH)
                ]
                for nt in range(KT):
                    for lkt in range(KSH):
                        nc.tensor.matmul(
                            accs[lkt], wts_sb[:, nt, ts(lkt, P)], fcwT_sb[:, nt],
                            start=(nt == 0), stop=(nt == KT - 1),
                        )
                for lkt in range(KSH):
                    nc.vector.tensor_copy(wfc_stage[:, lkt], accs[lkt])

                # collective staging on the scalar HWDGE queue
                gin = ccd.tile([P, KSH * FC], BF16)
                nc.scalar.dma_start(gin, wfc_stage.rearrange("p a b -> p (a b)"))
                gout = ccd.tile([NCORES * P, KSH * FC], BF16, addr_space="Shared")
                nc.gpsimd.collective_compute(
                    "AllGather",
                    mybir.AluOpType.bypass,
                    replica_groups=[list(range(NCORES))],
                    ins=[gin.opt()],
                    outs=[gout.opt()],
                )
                # gout rows = (core c, partition p); core c's shard is
                # global k-tiles 3c..3c+2
                nc.scalar.dma_start(
                    wfc_sb.rearrange("p (c l) f -> p c (l f)", c=NCORES),
                    gout.rearrange("(c p) j -> p c j", p=P),
                )

                # xT chunks ride the gpsimd (SWDGE) queue BEHIND the
                # collective: the gather gets a quiet fabric, then the four
                # 3.15 MB chunks stream in while phase B consumes them
                for mc in range(MC):
                    nc.gpsimd.dma_start(
                        xts[mc].rearrange("p t m -> p (t m)"),
                        xT_d[:, ds(mc * KT * MCH, KT * MCH)],
                    )

            # ------------ Phase B: h2T = relu(wfc.T @ xT + b) ------------
            with (
                tc.tile_pool(name="ps_b", bufs=4, space=MemorySpace.PSUM) as ps_b,
                tc.tile_pool(name="ps_lg", bufs=2, space=MemorySpace.PSUM) as ps_lg,
            ):
                out_sb = out_pool.tile([CLS, BS], F32)
                for mc in range(MC):
                    yt = yt_pool.tile([P, FT, MCH], BF16, tag="yt")
                    for ft in range(FT):
                        ps = ps_b.tile([P, MCH], F32, tag="h2", name=f"h2_{mc}_{ft}")
                        for kt in range(KT):
                            nc.tensor.matmul(
                                ps,
                                wfc_sb[:, kt, ts(ft, P)],
                                xts[mc][:, kt],
                                start=(kt == 0),
                                stop=(kt == KT - 1),
                            )
                        nc.scalar.activation(
                            yt[:, ft],
                            ps,
                            mybir.ActivationFunctionType.Relu,
                            bias=fcb_sb[:, ds(ft, 1)],
                        )
                    # logits for this m-chunk; overlaps next chunk's matmuls
                    plg = ps_lg.tile([CLS, MCH], F32, tag="lg")
                    for ft in range(FT):
                        nc.tensor.matmul(
                            plg,
                            lgwT_sb[:, ft],
                            yt[:, ft],
                            start=(ft == 0),
                            stop=False,
                        )
                    nc.tensor.matmul(plg, lgb_sb, ones_sb, start=False, stop=True)
                    nc.vector.tensor_copy(out_sb[:, ts(mc, MCH)], plg)

                nc.sync.dma_start(out_d, out_sb)

    nc.compile()
    return nc


def _permute(a2d, rows_per_tile=P):
    """[T*P, F] -> [P, T*F] so partition p's data is contiguous in DRAM."""
    t = a2d.shape[0] // rows_per_tile
    return np.ascontiguousarray(
        a2d.reshape(t, rows_per_tile, a2d.shape[1])
        .transpose(1, 0, 2)
        .reshape(rows_per_tile, t * a2d.shape[1])
    )


def prep_inputs(inputs):
    """Host-side layout marshaling: slice per core, pre-transpose, bf16."""
    x = np.asarray(inputs["x"], dtype=np.float32)
    W = np.asarray(inputs["W"], dtype=np.float32)
    fc_w = np.asarray(inputs["fc_w"], dtype=np.float32)
    fc_b = np.ascontiguousarray(inputs["fc_b"], dtype=np.float32)
    lgw = np.asarray(inputs["logits_w"], dtype=np.float32)
    lgb = np.asarray(inputs["logits_b"], dtype=np.float32)

    xT = x.astype(BF).T                              # [N, B] view
    WT = W.astype(BF).T                              # [N, N] rows=n, cols=k
    fcwTr = _permute(np.ascontiguousarray(fc_w.astype(BF).T))  # [P, KT*FC]
    lgwT = np.ascontiguousarray(lgw.astype(BF).T)    # [FC, CLS]
    lgb_bf = lgb.astype(BF)

    in_maps = []
    for i in range(NCORES):
        # mc-major xT: MC contiguous blocks of [P, KT*MCH]
        xblocks = [
            _permute(np.ascontiguousarray(
                xT[:, i * BS + mc * MCH : i * BS + (mc + 1) * MCH]
            ))
            for mc in range(MC)
        ]
        m = {
            "xTr": np.ascontiguousarray(np.concatenate(xblocks, axis=1)),
            "WTsr": _permute(np.ascontiguousarray(WT[:, i * WK : (i + 1) * WK])),
            "fcwTr": fcwTr,
            "fc_b": fc_b,
            "lgwT": lgwT,
            "lgb": lgb_bf,
        }
        in_maps.append(m)
    return in_maps


def kernel(**inputs) -> np.ndarray:
    global LAST_RESULT
    if "nc" not in _CACHE:
        _CACHE["nc"] = build_kernel()
    nc = _CACHE["nc"]

    in_maps = prep_inputs(inputs)
    res = run_bass_kernel_spmd(
        nc,
        in_maps,
        core_ids=list(range(NCORES)),
        trace=bool(int(os.environ.get("KERNEL_TRACE", "0"))),
    )
    LAST_RESULT = res
    # per-core out is [CLS, BS]; transpose back to [BS, CLS]
    out = np.concatenate(
        [np.ascontiguousarray(r_["out"].T) for r_ in res.results], axis=0
    )
    return out


# revision 10
# speedup vs baseline: 1.1335x; 1.1335x over previous
"""Trainium2 Bass kernel for nn_LDRFat (3-layer MLP forward).

reference: logits = relu((x @ W) @ fc_w.T + fc_b) @ logits_w.T + logits_b

Algebraic optimization: (x @ W) @ fc_w.T == x @ (W @ fc_w.T).
Precomputing Wfc = W @ fc_w.T ([3072,512]) collapses the dominant
309-GFLOP x@W matmul into a 51.5-GFLOP x@Wfc (phase B).

Device strategy (all matmul operands bf16, f32 accumulate):
 - Host feeds pre-transposed, pre-permuted operands laid out exactly as
   the SBUF tiles ([partition, free] contiguous), so every DMA moves
   large contiguous per-partition chunks at full HBM bandwidth and the
   device does ZERO transposes.
 - Phase A (sharded over W's k-rows, 3 k-tiles/core): wfc_shard =
   W[kshard,:] @ fc_w.T via 72 MMs, nt-outer with 3 open PSUM groups so
   compute starts after the first half-chunk of its inputs lands.
   Shards combined with a bf16 AllGather (staging DMAs on the scalar
   HWDGE queue so they never sit behind the big xT DMA on sync).
 - Phase B (data-parallel over batch, 2048 rows/core): h2T[f,m] =
   wfc-tiles (stationary) x xT (moving, free=512), ft-outer / kt / mc
   so each stationary tile serves 4 matmuls. ACT applies fused
   bias+relu. Logits computed as outT[cls,m] with logits_wT stationary
   and bias added via a K=1 ones x logits_b matmul; host transposes the
   [10, 2048] per-core result back.
"""

import os
import numpy as np
import ml_dtypes

import concourse.bass as bass
import concourse.mybir as mybir
import concourse.tile as tile
from concourse import bacc
from concourse.bass import MemorySpace, ts, ds
from concourse.bass_utils import run_bass_kernel_spmd

B = 16384
N = 3072
FC = 512
CLS = 10
NCORES = 8
BS = B // NCORES     # 2048 batch rows per core
P = 128

KT = N // P          # 24 k/n tiles
FT = FC // P         # 4 f-tiles
MC = 4               # m-chunks per core
MCH = BS // MC       # 512
KSH = KT // NCORES   # 3 k-tiles per core in sharded precompute
WK = KSH * P         # 384 W-rows per core
NCHUNK = 2           # phase-A input DMA chunks
NTC = KT // NCHUNK   # nt-tiles per chunk

F32 = mybir.dt.float32
BF16 = mybir.dt.bfloat16
BF = ml_dtypes.bfloat16

_CACHE = {}
LAST_RESULT = None


def build_kernel():
    nc = bacc.Bacc(
        "TRN2",
        target_bir_lowering=False,
        debug=False,
        enable_asserts=False,
        num_devices=NCORES,
    )
    # pre-permuted [partition, free] layouts (see prep_inputs)
    xT_d = nc.dram_tensor("xTr", [P, KT * BS], BF16, kind="ExternalInput").ap()
    wts_d = nc.dram_tensor("WTsr", [P, KT * WK], BF16, kind="ExternalInput").ap()
    fcwT_d = nc.dram_tensor("fcwTr", [P, KT * FC], BF16, kind="ExternalInput").ap()
    fcb_d = nc.dram_tensor("fc_b", [FC], F32, kind="ExternalInput").ap()
    lgwT_d = nc.dram_tensor("lgwT", [FC, CLS], BF16, kind="ExternalInput").ap()
    lgb_d = nc.dram_tensor("lgb", [CLS], BF16, kind="ExternalInput").ap()
    out_d = nc.dram_tensor("out", [CLS, BS], F32, kind="ExternalOutput").ap()

    with tile.TileContext(nc) as tc:
        with (
            tc.tile_pool(name="consts", bufs=1) as consts,
            tc.tile_pool(name="wfc", bufs=1) as wfc_pool,
            tc.tile_pool(name="xt", bufs=1) as xt_pool,
            tc.tile_pool(name="yt", bufs=1) as yt_pool,
            tc.tile_pool(name="osb", bufs=1) as out_pool,
        ):
            # ---- constants / small inputs (issued first on sync queue) ----
            fcb_sb = consts.tile([P, FT], F32)
            nc.sync.dma_start(fcb_sb, fcb_d.rearrange("(t p) -> p t", p=P))
            lgwT_sb = consts.tile([P, FT, CLS], BF16)
            nc.sync.dma_start(lgwT_sb, lgwT_d.rearrange("(t p) c -> p t c", p=P))
            lgb_sb = consts.tile([1, CLS], BF16)
            nc.sync.dma_start(lgb_sb, lgb_d.rearrange("(a c) -> a c", a=1))
            ones_stage = consts.tile([1, MCH], F32)
            nc.gpsimd.memset(ones_stage, 1.0)
            ones_sb = consts.tile([1, MCH], BF16)
            nc.vector.tensor_copy(ones_sb, ones_stage)

            # ---- bulk inputs: phase-A operands chunked first, then xT ----
            wts_sb = consts.tile([P, KT, WK], BF16)
            fcwT_sb = consts.tile([P, KT, FC], BF16)
            for c in range(NCHUNK):
                nc.sync.dma_start(
                    wts_sb[:, ts(c, NTC)].rearrange("p t k -> p (t k)"),
                    wts_d[:, ds(c * NTC * WK, NTC * WK)],
                )
                nc.sync.dma_start(
                    fcwT_sb[:, ts(c, NTC)].rearrange("p t f -> p (t f)"),
                    fcwT_d[:, ds(c * NTC * FC, NTC * FC)],
                )
            xt_sb = xt_pool.tile([P, KT, BS], BF16)
            nc.sync.dma_start(
                xt_sb.rearrange("p t m -> p (t m)"), xT_d
            )

            # wfc[k, f] resident for all of phase B
            wfc_sb = wfc_pool.tile([P, KT, FC], BF16)

            # ---------------- Phase A: wfc = W @ fc_w.T ----------------
            with (
                tc.tile_pool(name="wstg", bufs=1) as stage_pool,
                tc.tile_pool(name="ps_a", bufs=3, space=MemorySpace.PSUM) as ps_a,
                tc.tile_pool(name="ccd", bufs=1, space=MemorySpace.DRAM) as ccd,
            ):
                wfc_stage = stage_pool.tile([P, KSH, FC], BF16)
                accs = [
                    ps_a.tile([P, FC], F32, tag="acc", name=f"acc{lkt}")
                    for lkt in range(KSH)
                ]
                for nt in range(KT):
                    for lkt in range(KSH):
                        nc.tensor.matmul(
                            accs[lkt], wts_sb[:, nt, ts(lkt, P)], fcwT_sb[:, nt],
                            start=(nt == 0), stop=(nt == KT - 1),
                        )
                for lkt in range(KSH):
                    nc.vector.tensor_copy(wfc_stage[:, lkt], accs[lkt])

                # collective staging on the scalar HWDGE queue: never queued
                # behind the 12.6 MB xT DMA on sync
                gin = ccd.tile([P, KSH * FC], BF16)
                nc.scalar.dma_start(gin, wfc_stage.rearrange("p a b -> p (a b)"))
                gout = ccd.tile([NCORES * P, KSH * FC], BF16, addr_space="Shared")
                nc.gpsimd.collective_compute(
                    "AllGather",
                    mybir.AluOpType.bypass,
                    replica_groups=[list(range(NCORES))],
                    ins=[gin.opt()],
                    outs=[gout.opt()],
                )
                # gout rows = (core c, partition p); core c's shard is
                # global k-tiles 3c..3c+2
                nc.scalar.dma_start(
                    wfc_sb.rearrange("p (c l) f -> p c (l f)", c=NCORES),
                    gout.rearrange("(c p) j -> p c j", p=P),
                )

            # ------------ Phase B: h2T = relu(wfc.T @ xT + b) ------------
            with (
                tc.tile_pool(name="ps_b", bufs=6, space=MemorySpace.PSUM) as ps_b,
                tc.tile_pool(name="ps_lg", bufs=2, space=MemorySpace.PSUM) as ps_lg,
            ):
                out_sb = out_pool.tile([CLS, BS], F32)
                yts = []
                for ft in range(FT):
                    # 4 open accumulation groups; stationary wfc tile serves
                    # the 4 m-chunks
                    ps = [
                        ps_b.tile([P, MCH], F32, tag="h2", name=f"h2_{ft}_{mc}")
                        for mc in range(MC)
                    ]
                    for kt in range(KT):
                        for mc in range(MC):
                            nc.tensor.matmul(
                                ps[mc],
                                wfc_sb[:, kt, ts(ft, P)],
                                xt_sb[:, kt, ts(mc, MCH)],
                                start=(kt == 0),
                                stop=(kt == KT - 1),
                            )
                    yt = yt_pool.tile([P, MC, MCH], BF16, tag=f"yt{ft}")
                    for mc in range(MC):
                        nc.scalar.activation(
                            yt[:, mc],
                            ps[mc],
                            mybir.ActivationFunctionType.Relu,
                            bias=fcb_sb[:, ds(ft, 1)],
                        )
                    yts.append(yt)

                # logits: outT[cls, m] per m-chunk; stationary = lgwT tiles
                for mc in range(MC):
                    plg = ps_lg.tile([CLS, MCH], F32, tag="lg")
                    for ft in range(FT):
                        nc.tensor.matmul(
                            plg,
                            lgwT_sb[:, ft],
                            yts[ft][:, mc],
                            start=(ft == 0),
                            stop=False,
                        )
                    nc.tensor.matmul(plg, lgb_sb, ones_sb, start=False, stop=True)
                    nc.vector.tensor_copy(out_sb[:, ts(mc, MCH)], plg)

                nc.sync.dma_start(out_d, out_sb)

    nc.compile()
    return nc


def _permute(a2d, rows_per_tile=P):
    """[T*P, F] -> [P, T*F] so partition p's data is contiguous in DRAM."""
    t = a2d.shape[0] // rows_per_tile
    return np.ascontiguousarray(
        a2d.reshape(t, rows_per_tile, a2d.shape[1])
        .transpose(1, 0, 2)
        .reshape(rows_per_tile, t * a2d.shape[1])
    )


def prep_inputs(inputs):
    """Host-side layout marshaling: slice per core, pre-transpose, bf16."""
    x = np.asarray(inputs["x"], dtype=np.float32)
    W = np.asarray(inputs["W"], dtype=np.float32)
    fc_w = np.asarray(inputs["fc_w"], dtype=np.float32)
    fc_b = np.ascontiguousarray(inputs["fc_b"], dtype=np.float32)
    lgw = np.asarray(inputs["logits_w"], dtype=np.float32)
    lgb = np.asarray(inputs["logits_b"], dtype=np.float32)

    xT = x.astype(BF).T                              # [N, B] view
    WT = W.astype(BF).T                              # [N, N] rows=n, cols=k
    fcwTr = _permute(np.ascontiguousarray(fc_w.astype(BF).T))  # [P, KT*FC]
    lgwT = np.ascontiguousarray(lgw.astype(BF).T)    # [FC, CLS]
    lgb_bf = lgb.astype(BF)

    in_maps = []
    for i in range(NCORES):
        m = {
            "xTr": _permute(np.ascontiguousarray(xT[:, i * BS : (i + 1) * BS])),
            "WTsr": _permute(np.ascontiguousarray(WT[:, i * WK : (i + 1) * WK])),
            "fcwTr": fcwTr,
            "fc_b": fc_b,
            "lgwT": lgwT,
            "lgb": lgb_bf,
        }
        in_maps.append(m)
    return in_maps


def kernel(**inputs) -> np.ndarray:
    global LAST_RESULT
    if "nc" not in _CACHE:
        _CACHE["nc"] = build_kernel()
    nc = _CACHE["nc"]

    in_maps = prep_inputs(inputs)
    res = run_bass_kernel_spmd(
        nc,
        in_maps,
        core_ids=list(range(NCORES)),
        trace=bool(int(os.environ.get("KERNEL_TRACE", "0"))),
    )
    LAST_RESULT = res
    # per-core out is [CLS, BS]; transpose back to [BS, CLS]
    out = np.concatenate(
        [np.ascontiguousarray(r_["out"].T) for r_ in res.results], axis=0
    )
    return out
